# revision 1
# baseline (speedup 1.0000x reference)
"""
CoordinationHistogram TRN2 kernel, v3: bin-major H masks.

Same two-level one-hot matmul as kernel.py, but the H one-hot is built
bin-major: per block of T columns, one tensor_scalar(q_tile, j, is_equal)
per bin j writes H for all T columns at once (58-cycle DVE overhead
amortized T-fold). The matmul reads H as a stride-T access pattern.
L masks stay per-column (they carry the per-edge weight z, which forces a
per-partition-scalar op); they are split DVE/GPSIMD. ACT builds the tail
bins via the exact integer one-hot Square -> Relu(1-x).
"""

import numpy as np

import concourse.tile as tile
from concourse import bacc, mybir
from concourse.bass_utils import run_bass_kernel_spmd

P = 128
NQ = 158
NATOMS = 20000
K = 16
E = 1_000_000
NCOL_FULL = 7813
TBLK = 208          # columns per block (bin-major window)
GRP = 22            # L-mask mega-tile group
DVE_BINS = 125      # H bins built on DVE; rest (33) on ACT
GPS_L = 11          # L columns per GRP built on GPSIMD (rest on DVE)

R1 = 4.4
INV2 = float(1.0 / (1.1 * 1.1))
PAD_ATOM = 20064

F32 = mybir.dt.float32
BF16 = mybir.dt.bfloat16
I32 = mybir.dt.int32
OP = mybir.AluOpType
AF = mybir.ActivationFunctionType


def _emit_cols(nc, ncol, col, blk, iota_l, mpool, coords):
    """Emit L masks + matmuls for one (deferred) block."""
    tb, rf, zf, hview = blk
    t = 0
    grp_idx = 0
    while t < tb:
        g = min(GRP, tb - t)
        gps_l = GPS_L if (grp_idx % 2 == 0) else GPS_L - 1
        if g < GRP:
            gps_l = (g * (2 * GPS_L - 1) + 2) // (2 * GRP)
        grp_idx += 1
        lmega = mpool.tile([P, GRP * P], BF16, tag="lmega")
        for i in range(g):
            lslice = lmega[:, i * P:(i + 1) * P]
            eng = nc.gpsimd if i < gps_l else nc.vector
            eng.tensor_scalar(
                lslice, iota_l[:],
                rf[:, t + i:t + i + 1], zf[:, t + i:t + i + 1],
                op0=OP.is_equal, op1=OP.mult)
        for i in range(g):
            nc.tensor.matmul(
                out=coords[:],
                lhsT=lmega[:, i * P:(i + 1) * P],
                rhs=hview[:, :, t + i],
                start=(col + i == 0), stop=(col + i == ncol - 1))
        col += g
        t += g
    return col


def build_nc(ncol=NCOL_FULL):
    e_pad = ncol * P
    nc = bacc.Bacc("TRN2", target_bir_lowering=False, debug=False)
    nv = nc.dram_tensor("nv", [e_pad * 3], F32, kind="ExternalInput")
    fa = nc.dram_tensor("fa", [e_pad], I32, kind="ExternalInput")
    out = nc.dram_tensor("out", [1, K], F32, kind="ExternalOutput")

    blocks = []
    c = 0
    while c < ncol:
        tb = min(TBLK, ncol - c)
        blocks.append((c, tb))
        c += tb

    with tile.TileContext(nc) as tc:
        with (
            tc.tile_pool(name="const", bufs=1) as cpool,
            tc.tile_pool(name="io", bufs=2) as iopool,
            tc.tile_pool(name="work", bufs=2) as wpool,
            tc.tile_pool(name="hb", bufs=2) as hpool,
            tc.tile_pool(name="mask", bufs=4) as mpool,
            tc.tile_pool(name="psum", bufs=1, space="PSUM") as ppool,
        ):
            iota_l = cpool.tile([P, P], BF16)
            nc.gpsimd.iota(iota_l[:], pattern=[[1, P]], base=0,
                           channel_multiplier=0,
                           allow_small_or_imprecise_dtypes=True)
            ones = cpool.tile([P, 1], F32)
            nc.vector.memset(ones[:], 1.0)
            bias_m1 = cpool.tile([P, 1], F32)
            nc.vector.memset(bias_m1[:], -1.0)
            bias_m4 = cpool.tile([P, 1], F32)
            nc.vector.memset(bias_m4[:], -4.0)
            bias_k = cpool.tile([P, K], F32)
            for k in range(K):
                nc.vector.memset(bias_k[:, k:k + 1], float(-k))
            # -j bias table for ACT-built bins
            bias_q = cpool.tile([P, NQ], F32)
            iq = cpool.tile([P, NQ], mybir.dt.int16)
            nc.gpsimd.iota(iq[:], pattern=[[1, NQ]], base=0,
                           channel_multiplier=0)
            nc.vector.tensor_copy(bias_q[:], iq[:])
            nc.vector.tensor_scalar(bias_q[:], bias_q[:], -1.0, None,
                                    op0=OP.mult)

            coords = ppool.tile([P, NQ], F32, space="PSUM")

            col = 0
            prev = None
            for (c0, tb) in blocks:
                ofs_e = c0 * P
                nvb = iopool.tile([P, TBLK * 3], F32, tag="nvb")
                fab = iopool.tile([P, TBLK], I32, tag="fab")
                nc.sync.dma_start(
                    nvb[:, : tb * 3],
                    nv[ofs_e * 3: (ofs_e + P * tb) * 3].rearrange(
                        "(p m) -> p m", p=P),
                )
                nc.sync.dma_start(
                    fab[:, :tb],
                    fa[ofs_e: ofs_e + P * tb].rearrange("(p m) -> p m", p=P),
                )
                v3 = nvb[:, : tb * 3].rearrange("p (m c) -> p m c", c=3)
                x, y, w = v3[:, :, 0], v3[:, :, 1], v3[:, :, 2]

                d2 = wpool.tile([P, TBLK], F32, tag="d2")
                t1 = wpool.tile([P, TBLK], F32, tag="t1")
                nc.vector.tensor_tensor(out=d2[:, :tb], in0=x, in1=x, op=OP.mult)
                nc.vector.tensor_tensor(out=t1[:, :tb], in0=y, in1=y, op=OP.mult)
                nc.vector.tensor_tensor(out=d2[:, :tb], in0=d2[:, :tb],
                                        in1=t1[:, :tb], op=OP.add)
                nc.vector.tensor_tensor(out=t1[:, :tb], in0=w, in1=w, op=OP.mult)
                nc.vector.tensor_tensor(out=d2[:, :tb], in0=d2[:, :tb],
                                        in1=t1[:, :tb], op=OP.add)
                sv = wpool.tile([P, TBLK], F32, tag="sv")
                nc.scalar.activation(sv[:, :tb], d2[:, :tb], AF.Sqrt, scale=INV2)
                y0 = wpool.tile([P, TBLK], F32, tag="y0")
                nc.scalar.activation(y0[:, :tb], sv[:, :tb], AF.Relu,
                                     bias=bias_m4[:])
                yc = wpool.tile([P, TBLK], F32, tag="yc")
                nc.vector.tensor_scalar(yc[:, :tb], y0[:, :tb], 1.0, None,
                                        op0=OP.min)
                vv = wpool.tile([P, TBLK], F32, tag="vv")
                nc.scalar.activation(vv[:, :tb], yc[:, :tb], AF.Square,
                                     bias=bias_m1[:])
                w2 = wpool.tile([P, TBLK], F32, tag="w2")
                nc.vector.tensor_scalar(w2[:, :tb], yc[:, :tb], 2.0, 1.0,
                                        op0=OP.mult, op1=OP.add)
                zf = wpool.tile([P, TBLK], F32, tag="zf")
                nc.vector.tensor_tensor(out=zf[:, :tb], in0=vv[:, :tb],
                                        in1=w2[:, :tb], op=OP.mult)
                qi = wpool.tile([P, TBLK], I32, tag="qi")
                ri = wpool.tile([P, TBLK], I32, tag="ri")
                nc.vector.tensor_scalar(qi[:, :tb], fab[:, :tb], 7, None,
                                        op0=OP.logical_shift_right)
                nc.vector.tensor_scalar(ri[:, :tb], fab[:, :tb], 127, None,
                                        op0=OP.bitwise_and)
                qf = wpool.tile([P, TBLK], F32, tag="qf")
                rf = wpool.tile([P, TBLK], F32, tag="rf")
                nc.vector.tensor_copy(qf[:, :tb], qi[:, :tb])
                nc.vector.tensor_copy(rf[:, :tb], ri[:, :tb])
                qb = wpool.tile([P, TBLK], BF16, tag="qb")
                nc.vector.tensor_copy(qb[:, :tb], qf[:, :tb])

                # ---- software pipeline: emit L masks + matmuls for the
                # PREVIOUS block first (PE gets lhsT tiles early in this
                # engine section), then this block's bins — which therefore
                # complete a full block ahead of their matmuls and never
                # gate the PE.
                if prev is not None:
                    col = _emit_cols(nc, ncol, col, prev, iota_l, mpool, coords)

                # ---- bin-major H: HB[p, j*TBLK + t] = (q[p,t] == j) ----
                hb = hpool.tile([P, NQ * TBLK], BF16, tag="hb")
                for j in range(NQ):
                    hslice = hb[:, j * TBLK: j * TBLK + tb]
                    if j < DVE_BINS:
                        nc.vector.tensor_scalar(hslice, qb[:, :tb], float(j),
                                                None, op0=OP.is_equal)
                    else:
                        hsq = mpool.tile([P, TBLK], BF16, tag="hsq")
                        nc.scalar.activation(hsq[:, :tb], qb[:, :tb],
                                             AF.Square,
                                             bias=bias_q[:, j:j + 1])
                        nc.scalar.activation(hslice, hsq[:, :tb], AF.Relu,
                                             bias=ones[:], scale=-1.0)
                hview = hb[:].rearrange("p (j t) -> p j t", t=TBLK)
                prev = (tb, rf, zf, hview)
            col = _emit_cols(nc, ncol, col, prev, iota_l, mpool, coords)

            # ---- KDE ----
            acc1 = cpool.tile([P, K], F32)
            acc2 = cpool.tile([32, K], F32)
            sq = wpool.tile([P, NQ], F32, tag="sq")
            ek = wpool.tile([P, NQ], F32, tag="ek")
            for k in range(K):
                nc.scalar.activation(sq[:], coords[:], AF.Square,
                                     bias=bias_k[:, k:k + 1])
                nc.scalar.activation(ek[:, :156], sq[:, :156], AF.Exp,
                                     scale=-2.0, accum_out=acc1[:, k:k + 1])
                nc.scalar.activation(ek[0:32, 156:157], sq[0:32, 156:157],
                                     AF.Exp, scale=-2.0,
                                     accum_out=acc2[:, k:k + 1])
            hist_ps = ppool.tile([1, K], F32, space="PSUM")
            nc.tensor.matmul(out=hist_ps[:], lhsT=ones[:], rhs=acc1[:],
                             start=True, stop=False)
            nc.tensor.matmul(out=hist_ps[:], lhsT=ones[0:32, :], rhs=acc2[:],
                             start=False, stop=True)
            res = cpool.tile([1, K], F32)
            nc.vector.tensor_copy(res[:], hist_ps[:])
            nc.sync.dma_start(out[:], res[:])
    nc.compile()
    return nc


def _shard_inputs(neighbor_vectors, first_atom, ncol=NCOL_FULL):
    e_pad = ncol * P
    s = neighbor_vectors.shape[0]
    in_maps = []
    for i in range(s):
        nvs = np.asarray(neighbor_vectors[i], dtype=np.float32).reshape(-1, 3)
        fas = np.asarray(first_atom[i], dtype=np.int32).reshape(-1)
        n = min(e_pad, nvs.shape[0])
        nv_pad = np.empty((e_pad, 3), dtype=np.float32)
        nv_pad[:n] = nvs[:n]
        nv_pad[n:] = np.array([10.0, 0.0, 0.0], dtype=np.float32)
        fa_pad = np.full((e_pad,), PAD_ATOM, dtype=np.int32)
        fa_pad[:n] = fas[:n]
        in_maps.append({"nv": nv_pad.reshape(-1), "fa": fa_pad})
    return in_maps


def run(neighbor_vectors, first_atom, ncol=NCOL_FULL, trace=False):
    nc = build_nc(ncol)
    in_maps = _shard_inputs(neighbor_vectors, first_atom, ncol)
    br = run_bass_kernel_spmd(nc, in_maps, core_ids=list(range(len(in_maps))),
                              trace=trace)
    out = np.stack([br.results[i]["out"][0] for i in range(len(in_maps))])
    return out.astype(np.float32), br


def kernel(neighbor_vectors, first_atom):
    out, _ = run(neighbor_vectors, first_atom)
    return out



# revision 5
# speedup vs baseline: 4.5511x; 4.5511x over previous
"""
CoordinationHistogram TRN2 kernel, v5: partition-routed scatter.

Layout strategy (host side, part of the sharding/layout choice in
kernel()): each system's edges are placed so that partition p holds
exactly the edges whose atom index a has (a & 127) == p, ordered by
coarse group g = (a >> 7) // 16 (10 groups of 2048 atoms), padded per
(partition, group) to a static rectangular width.  On device the
scatter then needs NO per-edge r one-hot: the matmul lhsT is a static
identity (partition == atom low bits), and within group g the q one-hot
is only 16 wide (q - 16g in [0,16)).  Per column of 128 edges the
device does: bin-major 16-wide is_equal masks (DVE), a broadcast
multiply by z (DVE/Pool split), and one N=16 identity matmul
accumulating into PSUM coords[p, q].  KDE as in v3.
"""

import numpy as np
import ml_dtypes

import concourse.tile as tile
from concourse import bacc, mybir
from concourse.bass_utils import run_bass_kernel_spmd

P = 128
NG = 10              # coarse groups of 16 q-bins (2048 atoms)
NQ = 160             # coords free width (157 used)
K = 16
NATOMS = 20000
NCOL_FULL = 7813     # kept for test.py compat (full edges / 128)

R1 = 4.4
INV2 = float(1.0 / (1.1 * 1.1))
QPAD = 2000.0        # pad q value: matches no bin

F32 = mybir.dt.float32
BF16 = mybir.dt.bfloat16
I32 = mybir.dt.int32
OP = mybir.AluOpType
AF = mybir.ActivationFunctionType

BLK = 1024           # max columns per device block


def _blocks_for(lg):
    nb = (lg + BLK - 1) // BLK
    base = lg // nb
    rem = lg - base * nb
    return [base + (1 if i < rem else 0) for i in range(nb)]


def build_nc(lgs):
    """lgs: list of NG padded widths (columns per group)."""
    cols = sum(lgs)
    nc = bacc.Bacc("TRN2", target_bir_lowering=False, debug=False)
    nv = nc.dram_tensor("nv", [P * cols * 3], F32, kind="ExternalInput")
    qs = nc.dram_tensor("qs", [P * cols], BF16, kind="ExternalInput")
    out = nc.dram_tensor("out", [1, K], F32, kind="ExternalOutput")
    nvt = nv.rearrange("(p m) -> p m", p=P)
    qst = qs.rearrange("(p m) -> p m", p=P)

    with tile.TileContext(nc) as tc:
        with (
            tc.tile_pool(name="const", bufs=1) as cpool,
            tc.tile_pool(name="io", bufs=3) as iopool,
            tc.tile_pool(name="work", bufs=2) as wpool,
            tc.tile_pool(name="hb", bufs=2) as hpool,
            tc.tile_pool(name="psum", bufs=1, space="PSUM") as ppool,
        ):
            # static identity lhsT: ident[p, f] = (p == f)
            iota_row = cpool.tile([P, P], BF16)
            nc.gpsimd.iota(iota_row[:], pattern=[[1, P]], base=0,
                           channel_multiplier=0,
                           allow_small_or_imprecise_dtypes=True)
            pidx = cpool.tile([P, 1], mybir.dt.int16)
            nc.gpsimd.iota(pidx[:], pattern=[[0, 1]], base=0,
                           channel_multiplier=1)
            pidx_f = cpool.tile([P, 1], F32)
            nc.vector.tensor_copy(pidx_f[:], pidx[:])
            ident = cpool.tile([P, P], BF16)
            nc.vector.tensor_scalar(ident[:], iota_row[:], pidx_f[:], None,
                                    op0=OP.is_equal)
            zrow = cpool.tile([P, NQ], BF16)
            nc.vector.memset(zrow[:], 0.0)
            ones = cpool.tile([P, 1], F32)
            nc.vector.memset(ones[:], 1.0)
            bias_m1 = cpool.tile([P, 1], F32)
            nc.vector.memset(bias_m1[:], -1.0)
            bias_m4 = cpool.tile([P, 1], F32)
            nc.vector.memset(bias_m4[:], -4.0)
            bias_k = cpool.tile([P, K], F32)
            for k in range(K):
                nc.vector.memset(bias_k[:, k:k + 1], float(-k))

            coords = ppool.tile([P, NQ], F32, space="PSUM")
            # open the PSUM accumulation group, zeroing coords
            nc.tensor.matmul(out=coords[:], lhsT=ident[:], rhs=zrow[:],
                             start=True, stop=False)

            total_mm = sum(lgs)
            mm = 0
            c0 = 0
            for g in range(NG):
                nbin = 16 if g < NG - 1 else 13
                for tb in _blocks_for(lgs[g]):
                    nvb = iopool.tile([P, BLK * 3], F32, tag="nvb")
                    qb = iopool.tile([P, BLK], BF16, tag="qb")
                    nc.sync.dma_start(nvb[:, :tb * 3],
                                      nvt[:, c0 * 3:(c0 + tb) * 3])
                    nc.sync.dma_start(qb[:, :tb], qst[:, c0:c0 + tb])
                    v3 = nvb[:, :tb * 3].rearrange("p (m c) -> p m c", c=3)
                    x, y, w = v3[:, :, 0], v3[:, :, 1], v3[:, :, 2]

                    # ---- z switching function (d2 sums on Pool) ----
                    d2 = wpool.tile([P, BLK], F32, tag="d2")
                    t1 = wpool.tile([P, BLK], F32, tag="t1")
                    nc.gpsimd.tensor_tensor(out=d2[:, :tb], in0=x, in1=x,
                                            op=OP.mult)
                    nc.gpsimd.tensor_tensor(out=t1[:, :tb], in0=y, in1=y,
                                            op=OP.mult)
                    nc.gpsimd.tensor_tensor(out=d2[:, :tb], in0=d2[:, :tb],
                                            in1=t1[:, :tb], op=OP.add)
                    nc.gpsimd.tensor_tensor(out=t1[:, :tb], in0=w, in1=w,
                                            op=OP.mult)
                    nc.gpsimd.tensor_tensor(out=d2[:, :tb], in0=d2[:, :tb],
                                            in1=t1[:, :tb], op=OP.add)
                    sv = wpool.tile([P, BLK], F32, tag="sv")
                    nc.scalar.activation(sv[:, :tb], d2[:, :tb], AF.Sqrt,
                                         scale=INV2)
                    y0 = wpool.tile([P, BLK], F32, tag="y0")
                    nc.scalar.activation(y0[:, :tb], sv[:, :tb], AF.Relu,
                                         bias=bias_m4[:])
                    yc = wpool.tile([P, BLK], F32, tag="yc")
                    nc.vector.tensor_scalar(yc[:, :tb], y0[:, :tb], 1.0, None,
                                            op0=OP.min)
                    vv = wpool.tile([P, BLK], F32, tag="vv")
                    nc.scalar.activation(vv[:, :tb], yc[:, :tb], AF.Square,
                                         bias=bias_m1[:])
                    w2 = wpool.tile([P, BLK], F32, tag="w2")
                    nc.vector.tensor_scalar(w2[:, :tb], yc[:, :tb], 2.0, 1.0,
                                            op0=OP.mult, op1=OP.add)
                    zb = wpool.tile([P, BLK], BF16, tag="zb")
                    nc.vector.tensor_tensor(out=zb[:, :tb], in0=vv[:, :tb],
                                            in1=w2[:, :tb], op=OP.mult)

                    # ---- bin-major 16-wide one-hot, then fold z in ----
                    hb = hpool.tile([P, 16 * BLK], BF16, tag="hb")
                    for j in range(nbin):
                        nc.vector.tensor_scalar(
                            hb[:, j * BLK:j * BLK + tb], qb[:, :tb],
                            float(16 * g + j), None, op0=OP.is_equal)
                    # multiply by z: split bins DVE / Pool for balance
                    jd = 10 if nbin == 16 else 8
                    for j in range(nbin):
                        eng = nc.vector if j < jd else nc.gpsimd
                        eng.tensor_tensor(
                            out=hb[:, j * BLK:j * BLK + tb],
                            in0=hb[:, j * BLK:j * BLK + tb],
                            in1=zb[:, :tb], op=OP.mult)
                    hv = hb[:].rearrange("p (j t) -> p j t", t=BLK)

                    # ---- identity matmuls accumulate into coords ----
                    for t in range(tb):
                        mm += 1
                        nc.tensor.matmul(
                            out=coords[:, 16 * g:16 * g + nbin],
                            lhsT=ident[:],
                            rhs=hv[:, :nbin, t],
                            start=False, stop=(mm == total_mm))
                    c0 += tb

            # ---- KDE ----
            acc1 = cpool.tile([P, K], F32)
            acc2 = cpool.tile([32, K], F32)
            sq = wpool.tile([P, NQ], F32, tag="sq")
            ek = wpool.tile([P, NQ], F32, tag="ek")
            for k in range(K):
                nc.scalar.activation(sq[:], coords[:], AF.Square,
                                     bias=bias_k[:, k:k + 1])
                nc.scalar.activation(ek[:, :156], sq[:, :156], AF.Exp,
                                     scale=-2.0, accum_out=acc1[:, k:k + 1])
                nc.scalar.activation(ek[0:32, 156:157], sq[0:32, 156:157],
                                     AF.Exp, scale=-2.0,
                                     accum_out=acc2[:, k:k + 1])
            hist_ps = ppool.tile([1, K], F32, space="PSUM")
            nc.tensor.matmul(out=hist_ps[:], lhsT=ones[:], rhs=acc1[:],
                             start=True, stop=False)
            nc.tensor.matmul(out=hist_ps[:], lhsT=ones[0:32, :], rhs=acc2[:],
                             start=False, stop=True)
            res = cpool.tile([1, K], F32)
            nc.vector.tensor_copy(res[:], hist_ps[:])
            nc.sync.dma_start(out[:], res[:])
    nc.compile()
    return nc


def _route_system(a, nvs):
    """Route one system's edges: partition = a & 127, group = (a>>7)//16.

    Returns (pos_r, pos_c, g, order) where edge order[i] goes to
    [pos_r[i], pos_c[i]] once per-group widths are fixed; here we return
    per-(r,g) counts and the sorted order for later placement."""
    r = a & 127
    q = a >> 7
    g = q // 16
    key = r * NG + g
    order = np.argsort(key, kind="stable")
    counts = np.bincount(key, minlength=P * NG).reshape(P, NG)
    return order, counts, r, q, g


def _shard_inputs(neighbor_vectors, first_atom):
    s = neighbor_vectors.shape[0]
    routed = []
    all_counts = []
    for i in range(s):
        a = np.asarray(first_atom[i], dtype=np.int64).reshape(-1)
        nvs = np.asarray(neighbor_vectors[i], dtype=np.float32).reshape(-1, 3)
        order, counts, r, q, g = _route_system(a, nvs)
        routed.append((order, counts, r, q, g, nvs))
        all_counts.append(counts)
    # static per-group widths: max over systems and partitions
    lgs = np.max(np.stack(all_counts), axis=(0, 1)).astype(np.int64)
    lgs = [int(v) for v in lgs]
    cols = sum(lgs)
    col_base = np.concatenate([[0], np.cumsum(lgs)])[:NG]

    in_maps = []
    for (order, counts, r, q, g, nvs) in routed:
        nv_pad = np.empty((P, cols, 3), dtype=np.float32)
        nv_pad[:, :, 0] = 10.0
        nv_pad[:, :, 1] = 0.0
        nv_pad[:, :, 2] = 0.0
        q_pad = np.full((P, cols), QPAD, dtype=np.float32)
        # within-group rank for each edge, in sorted order
        key_sorted_counts = counts.reshape(-1)
        starts = np.concatenate([[0], np.cumsum(key_sorted_counts)])[:-1]
        rank = np.arange(len(order), dtype=np.int64) - np.repeat(
            starts, key_sorted_counts)
        rs = r[order]
        gs_ = g[order]
        colpos = col_base[gs_] + rank
        nv_pad[rs, colpos] = nvs[order]
        q_pad[rs, colpos] = q[order].astype(np.float32)
        in_maps.append({
            "nv": nv_pad.reshape(-1),
            "qs": q_pad.astype(ml_dtypes.bfloat16).reshape(-1),
        })
    return in_maps, lgs


LAST_NC = None


def run(neighbor_vectors, first_atom, ncol=NCOL_FULL, trace=False):
    global LAST_NC
    in_maps, lgs = _shard_inputs(neighbor_vectors, first_atom)
    nc = build_nc(lgs)
    LAST_NC = nc
    br = run_bass_kernel_spmd(nc, in_maps, core_ids=list(range(len(in_maps))),
                              trace=trace)
    out = np.stack([br.results[i]["out"][0] for i in range(len(in_maps))])
    return out.astype(np.float32), br


def kernel(neighbor_vectors, first_atom):
    out, _ = run(neighbor_vectors, first_atom)
    return out


# revision 6
# speedup vs baseline: 7.8867x; 1.7329x over previous
"""
CoordinationHistogram TRN2 kernel, v5: partition-routed scatter.

Layout strategy (host side, part of the sharding/layout choice in
kernel()): each system's edges are placed so that partition p holds
exactly the edges whose atom index a has (a & 127) == p, ordered by
coarse group g = (a >> 7) // 16 (10 groups of 2048 atoms), padded per
(partition, group) to a static rectangular width.  On device the
scatter then needs NO per-edge r one-hot: the matmul lhsT is a static
identity (partition == atom low bits), and within group g the q one-hot
is only 16 wide (q - 16g in [0,16)).  Per column of 128 edges the
device does: bin-major 16-wide is_equal masks (DVE), a broadcast
multiply by z (DVE/Pool split), and one N=16 identity matmul
accumulating into PSUM coords[p, q].  KDE as in v3.
"""

import numpy as np
import ml_dtypes

import concourse.tile as tile
from concourse import bacc, mybir
from concourse.bass_utils import run_bass_kernel_spmd

P = 128
NG = 20              # coarse groups of 8 q-bins (1024 atoms)
NQ = 160             # coords free width (157 used)
K = 16
NATOMS = 20000
NCOL_FULL = 7813     # kept for test.py compat (full edges / 128)

R1 = 4.4
INV2 = float(1.0 / (1.1 * 1.1))
QPAD = 2000.0        # pad q value: matches no bin

F32 = mybir.dt.float32
BF16 = mybir.dt.bfloat16
I32 = mybir.dt.int32
OP = mybir.AluOpType
AF = mybir.ActivationFunctionType

BLK = 1024           # max columns per device block


def _blocks_for(lg):
    nb = (lg + BLK - 1) // BLK
    base = lg // nb
    rem = lg - base * nb
    return [base + (1 if i < rem else 0) for i in range(nb)]


def build_nc(lgs):
    """lgs: list of NG padded widths (columns per group)."""
    cols = sum(lgs)
    nc = bacc.Bacc("TRN2", target_bir_lowering=False, debug=False)
    nv = nc.dram_tensor("nv", [P * cols * 3], F32, kind="ExternalInput")
    qs = nc.dram_tensor("qs", [P * cols], BF16, kind="ExternalInput")
    out = nc.dram_tensor("out", [1, K], F32, kind="ExternalOutput")
    nvt = nv.rearrange("(p m) -> p m", p=P)
    qst = qs.rearrange("(p m) -> p m", p=P)

    with tile.TileContext(nc) as tc:
        with (
            tc.tile_pool(name="const", bufs=1) as cpool,
            tc.tile_pool(name="io", bufs=3) as iopool,
            tc.tile_pool(name="work", bufs=2) as wpool,
            tc.tile_pool(name="hb", bufs=2) as hpool,
            tc.tile_pool(name="psum", bufs=1, space="PSUM") as ppool,
        ):
            # static identity lhsT: ident[p, f] = (p == f)
            iota_row = cpool.tile([P, P], BF16)
            nc.gpsimd.iota(iota_row[:], pattern=[[1, P]], base=0,
                           channel_multiplier=0,
                           allow_small_or_imprecise_dtypes=True)
            pidx = cpool.tile([P, 1], mybir.dt.int16)
            nc.gpsimd.iota(pidx[:], pattern=[[0, 1]], base=0,
                           channel_multiplier=1)
            pidx_f = cpool.tile([P, 1], F32)
            nc.vector.tensor_copy(pidx_f[:], pidx[:])
            ident = cpool.tile([P, P], BF16)
            nc.vector.tensor_scalar(ident[:], iota_row[:], pidx_f[:], None,
                                    op0=OP.is_equal)
            zrow = cpool.tile([P, NQ], BF16)
            nc.vector.memset(zrow[:], 0.0)
            ones = cpool.tile([P, 1], F32)
            nc.vector.memset(ones[:], 1.0)
            bias_m1 = cpool.tile([P, 1], F32)
            nc.vector.memset(bias_m1[:], -1.0)
            bias_m4 = cpool.tile([P, 1], F32)
            nc.vector.memset(bias_m4[:], -4.0)
            bias_k = cpool.tile([P, K], F32)
            for k in range(K):
                nc.vector.memset(bias_k[:, k:k + 1], float(-k))

            coords = ppool.tile([P, NQ], F32, space="PSUM")
            # open the PSUM accumulation group, zeroing coords
            nc.tensor.matmul(out=coords[:], lhsT=ident[:], rhs=zrow[:],
                             start=True, stop=False)

            total_mm = sum(lgs)
            mm = 0
            c0 = 0
            for g in range(NG):
                nbin = 8 if g < NG - 1 else 5
                for tb in _blocks_for(lgs[g]):
                    nvb = iopool.tile([P, BLK * 3], F32, tag="nvb")
                    qb = iopool.tile([P, BLK], BF16, tag="qb")
                    nc.sync.dma_start(nvb[:, :tb * 3],
                                      nvt[:, c0 * 3:(c0 + tb) * 3])
                    nc.sync.dma_start(qb[:, :tb], qst[:, c0:c0 + tb])
                    v3 = nvb[:, :tb * 3].rearrange("p (m c) -> p m c", c=3)
                    x, y, w = v3[:, :, 0], v3[:, :, 1], v3[:, :, 2]

                    # ---- z switching function (squares on ACT, adds on Pool) ----
                    d2 = wpool.tile([P, BLK], F32, tag="d2")
                    t1 = wpool.tile([P, BLK], F32, tag="t1")
                    t2 = wpool.tile([P, BLK], F32, tag="t2")
                    nc.scalar.activation(t1[:, :tb], x, AF.Square)
                    nc.scalar.activation(d2[:, :tb], y, AF.Square)
                    nc.scalar.activation(t2[:, :tb], w, AF.Square)
                    nc.gpsimd.tensor_tensor(out=d2[:, :tb], in0=d2[:, :tb],
                                            in1=t1[:, :tb], op=OP.add)
                    nc.gpsimd.tensor_tensor(out=d2[:, :tb], in0=d2[:, :tb],
                                            in1=t2[:, :tb], op=OP.add)
                    sv = wpool.tile([P, BLK], F32, tag="sv")
                    nc.scalar.activation(sv[:, :tb], d2[:, :tb], AF.Sqrt,
                                         scale=INV2)
                    y0 = wpool.tile([P, BLK], F32, tag="y0")
                    nc.scalar.activation(y0[:, :tb], sv[:, :tb], AF.Relu,
                                         bias=bias_m4[:])
                    yc = wpool.tile([P, BLK], F32, tag="yc")
                    nc.vector.tensor_scalar(yc[:, :tb], y0[:, :tb], 1.0, None,
                                            op0=OP.min)
                    vv = wpool.tile([P, BLK], F32, tag="vv")
                    nc.scalar.activation(vv[:, :tb], yc[:, :tb], AF.Square,
                                         bias=bias_m1[:])
                    w2 = wpool.tile([P, BLK], F32, tag="w2")
                    nc.vector.tensor_scalar(w2[:, :tb], yc[:, :tb], 2.0, 1.0,
                                            op0=OP.mult, op1=OP.add)
                    zb = wpool.tile([P, BLK], BF16, tag="zb")
                    nc.vector.tensor_tensor(out=zb[:, :tb], in0=vv[:, :tb],
                                            in1=w2[:, :tb], op=OP.mult)

                    # ---- bin-major 16-wide one-hot, then fold z in ----
                    hb = hpool.tile([P, 8 * BLK], BF16, tag="hb")
                    for j in range(nbin):
                        nc.vector.tensor_scalar(
                            hb[:, j * BLK:j * BLK + tb], qb[:, :tb],
                            float(8 * g + j), None, op0=OP.is_equal)
                    # multiply by z: split bins DVE / Pool for balance
                    jd = 6 if nbin == 8 else 4
                    for j in range(nbin):
                        eng = nc.vector if j < jd else nc.gpsimd
                        eng.tensor_tensor(
                            out=hb[:, j * BLK:j * BLK + tb],
                            in0=hb[:, j * BLK:j * BLK + tb],
                            in1=zb[:, :tb], op=OP.mult)
                    hv = hb[:].rearrange("p (j t) -> p j t", t=BLK)

                    # ---- identity matmuls accumulate into coords ----
                    for t in range(tb):
                        mm += 1
                        nc.tensor.matmul(
                            out=coords[:, 8 * g:8 * g + nbin],
                            lhsT=ident[:],
                            rhs=hv[:, :nbin, t],
                            start=False, stop=(mm == total_mm))
                    c0 += tb

            # ---- KDE ----
            acc1 = cpool.tile([P, K], F32)
            acc2 = cpool.tile([32, K], F32)
            sq = wpool.tile([P, NQ], F32, tag="sq")
            ek = wpool.tile([P, NQ], F32, tag="ek")
            for k in range(K):
                nc.scalar.activation(sq[:], coords[:], AF.Square,
                                     bias=bias_k[:, k:k + 1])
                nc.scalar.activation(ek[:, :156], sq[:, :156], AF.Exp,
                                     scale=-2.0, accum_out=acc1[:, k:k + 1])
                nc.scalar.activation(ek[0:32, 156:157], sq[0:32, 156:157],
                                     AF.Exp, scale=-2.0,
                                     accum_out=acc2[:, k:k + 1])
            hist_ps = ppool.tile([1, K], F32, space="PSUM")
            nc.tensor.matmul(out=hist_ps[:], lhsT=ones[:], rhs=acc1[:],
                             start=True, stop=False)
            nc.tensor.matmul(out=hist_ps[:], lhsT=ones[0:32, :], rhs=acc2[:],
                             start=False, stop=True)
            res = cpool.tile([1, K], F32)
            nc.vector.tensor_copy(res[:], hist_ps[:])
            nc.sync.dma_start(out[:], res[:])
    nc.compile()
    return nc


def _route_system(a, nvs):
    """Route one system's edges: partition = a & 127, group = (a>>7)//16.

    Returns (pos_r, pos_c, g, order) where edge order[i] goes to
    [pos_r[i], pos_c[i]] once per-group widths are fixed; here we return
    per-(r,g) counts and the sorted order for later placement."""
    r = a & 127
    q = a >> 7
    g = q // 8
    key = r * NG + g
    order = np.argsort(key, kind="stable")
    counts = np.bincount(key, minlength=P * NG).reshape(P, NG)
    return order, counts, r, q, g


def _shard_inputs(neighbor_vectors, first_atom):
    s = neighbor_vectors.shape[0]
    routed = []
    all_counts = []
    for i in range(s):
        a = np.asarray(first_atom[i], dtype=np.int64).reshape(-1)
        nvs = np.asarray(neighbor_vectors[i], dtype=np.float32).reshape(-1, 3)
        order, counts, r, q, g = _route_system(a, nvs)
        routed.append((order, counts, r, q, g, nvs))
        all_counts.append(counts)
    # static per-group widths: max over systems and partitions
    lgs = np.max(np.stack(all_counts), axis=(0, 1)).astype(np.int64)
    lgs = [int(v) for v in lgs]
    cols = sum(lgs)
    col_base = np.concatenate([[0], np.cumsum(lgs)])[:NG]

    in_maps = []
    for (order, counts, r, q, g, nvs) in routed:
        nv_pad = np.empty((P, cols, 3), dtype=np.float32)
        nv_pad[:, :, 0] = 10.0
        nv_pad[:, :, 1] = 0.0
        nv_pad[:, :, 2] = 0.0
        q_pad = np.full((P, cols), QPAD, dtype=np.float32)
        # within-group rank for each edge, in sorted order
        key_sorted_counts = counts.reshape(-1)
        starts = np.concatenate([[0], np.cumsum(key_sorted_counts)])[:-1]
        rank = np.arange(len(order), dtype=np.int64) - np.repeat(
            starts, key_sorted_counts)
        rs = r[order]
        gs_ = g[order]
        colpos = col_base[gs_] + rank
        nv_pad[rs, colpos] = nvs[order]
        q_pad[rs, colpos] = q[order].astype(np.float32)
        in_maps.append({
            "nv": nv_pad.reshape(-1),
            "qs": q_pad.astype(ml_dtypes.bfloat16).reshape(-1),
        })
    return in_maps, lgs


LAST_NC = None


def run(neighbor_vectors, first_atom, ncol=NCOL_FULL, trace=False):
    global LAST_NC
    in_maps, lgs = _shard_inputs(neighbor_vectors, first_atom)
    nc = build_nc(lgs)
    LAST_NC = nc
    br = run_bass_kernel_spmd(nc, in_maps, core_ids=list(range(len(in_maps))),
                              trace=trace)
    out = np.stack([br.results[i]["out"][0] for i in range(len(in_maps))])
    return out.astype(np.float32), br


def kernel(neighbor_vectors, first_atom):
    out, _ = run(neighbor_vectors, first_atom)
    return out
